# revision 16
# baseline (speedup 1.0000x reference)
"""CEP loss kernel for Trainium2: loss = -sum(d1 * log(d2 + eps)).

Full inputs [4096, 4096] f32 are sharded row-wise across 8 NeuronCores
(512 rows each).  Per core the shard streams as 8 pieces of [128, 2048]
(1 MiB DMAs on the HWDGE queue):
  - ScalarE: t2 <- ln(d2 + eps) in place (+eps rides the activation bias)
  - VectorE: t2 <- d1 * t2 in place (fp32 tensor_tensor, 1x)
  - reduce along free dim to acc[:, piece], alternating engines per piece
    (even: ScalarE activation-Copy accum_out, odd: VectorE tensor_reduce)
    so neither engine becomes the critical path
Each core DMAs its [128, 8] partial-sum tile out; the host sums and
negates.  DMA (~16.8 MB/core at ~360-410 GB/s observed) is the
bottleneck: ACT ~23us and DVE ~26us busy both fit inside the ~41us DMA
window, and the post-DMA tail is only one piece of TT+reduce (~4us).
"""

import numpy as np

import concourse.bacc as bacc
import concourse.mybir as mybir
import concourse.tile as tile
from concourse.bass_utils import run_bass_kernel_spmd

N = 4096
N_CORES = 8
ROWS_PER_CORE = N // N_CORES  # 512
P = 128
N_TILES = ROWS_PER_CORE // P  # 4 row groups
PIECE_FD = 4096  # max piece width == SBUF tile width
MM_FD = 512  # one PSUM bank of fp32
EPS = 1e-5

# (row_tile, col_start, width) pieces: steady-state full-width 4096
# (2 MiB DMAs), tapered at the end so the post-DMA compute tail is short
_PIECES = []
for _i in range(N_TILES):
    widths = [4096]
    if _i == N_TILES - 1:
        widths = [2048, 1024, 512, 512]
    _c = 0
    for _w in widths:
        _PIECES.append((_i, _c, _w))
        _c += _w
    assert _c == N
N_PIECES = len(_PIECES)

_NC_CACHE = {}


def _build_nc():
    nc = bacc.Bacc(
        "TRN2", target_bir_lowering=False, debug=False, num_devices=N_CORES
    )
    d1 = nc.dram_tensor(
        "d1", [ROWS_PER_CORE, N], mybir.dt.float32, kind="ExternalInput"
    )
    d2 = nc.dram_tensor(
        "d2", [ROWS_PER_CORE, N], mybir.dt.float32, kind="ExternalInput"
    )
    out = nc.dram_tensor("partial", [1, 1], mybir.dt.float32, kind="ExternalOutput")
    d1t = d1.rearrange("(n p) m -> n p m", p=P)
    d2t = d2.rearrange("(n p) m -> n p m", p=P)

    with tile.TileContext(nc) as tc:
        with (
            tc.tile_pool(name="p1", bufs=4) as p1,
            tc.tile_pool(name="p2", bufs=4) as p2,
            tc.tile_pool(name="pprod", bufs=4) as pprod,
            tc.tile_pool(name="paux", bufs=1) as paux,
            tc.tile_pool(name="psum", bufs=1, space="PSUM") as psum_pool,
        ):
            bias = paux.tile([P, 1], mybir.dt.float32)
            nc.vector.memset(bias[:], EPS)
            ones = paux.tile([P, 1], mybir.dt.bfloat16)
            nc.vector.memset(ones[:], 1.0)
            colsum = psum_pool.tile([1, MM_FD], mybir.dt.float32)
            for k, (i, c0, w) in enumerate(_PIECES):
                fs = slice(c0, c0 + w)
                t1 = p1.tile([P, PIECE_FD], mybir.dt.float32, tag="t1")
                t2 = p2.tile([P, PIECE_FD], mybir.dt.float32, tag="t2")
                prod = pprod.tile([P, PIECE_FD], mybir.dt.bfloat16, tag="prod")
                nc.sync.dma_start(t2[:, :w], d2t[i][:, fs])
                nc.gpsimd.dma_start(t1[:, :w], d1t[i][:, fs])
                # t2 <- ln(d2 + eps), in place on ScalarE
                nc.scalar.activation(
                    t2[:, :w],
                    t2[:, :w],
                    mybir.ActivationFunctionType.Ln,
                    bias=bias[:, :],
                )
                # prod <- d1 * t2 on VectorE, cast to bf16 on the write
                nc.vector.tensor_mul(prod[:, :w], t1[:, :w], t2[:, :w])
                # column sums on the otherwise-idle TensorE (native bf16
                # matmul), every chunk accumulating into one PSUM bank
                for j in range(w // MM_FD):
                    nc.tensor.matmul(
                        colsum[:, :],
                        ones[:, 0:1],
                        prod[:, j * MM_FD : (j + 1) * MM_FD],
                        start=(k == 0 and j == 0),
                        stop=(k == N_PIECES - 1 and j == w // MM_FD - 1),
                    )
            total = paux.tile([1, 1], mybir.dt.float32)
            nc.vector.tensor_reduce(
                total[:, 0:1],
                colsum[:],
                axis=mybir.AxisListType.X,
                op=mybir.AluOpType.add,
            )
            nc.sync.dma_start(out[:], total[:])
    nc.compile()
    return nc


def _get_nc():
    if "nc" not in _NC_CACHE:
        _NC_CACHE["nc"] = _build_nc()
    return _NC_CACHE["nc"]


def run_spmd(in_maps, **kwargs):
    """Run the SPMD kernel; returns BassKernelResults (test harness passes
    trace=True kwargs for profiling)."""
    return run_bass_kernel_spmd(
        _get_nc(), in_maps, core_ids=list(range(N_CORES)), **kwargs
    )


def make_in_maps(distribution1, distribution2):
    d1 = np.asarray(distribution1, dtype=np.float32)
    d2 = np.asarray(distribution2, dtype=np.float32)
    in_maps = []
    for c in range(N_CORES):
        sl = slice(c * ROWS_PER_CORE, (c + 1) * ROWS_PER_CORE)
        in_maps.append(
            {
                "d1": np.ascontiguousarray(d1[sl]),
                "d2": np.ascontiguousarray(d2[sl]),
            }
        )
    return in_maps


def reduce_outputs(results):
    total = np.float64(0.0)
    for r in results:
        total += np.float64(r["partial"].sum(dtype=np.float64))
    return np.asarray([-total], dtype=np.float32)


def kernel(distribution1, distribution2):
    in_maps = make_in_maps(distribution1, distribution2)
    res = run_spmd(in_maps)
    return reduce_outputs(res.results)


# revision 17
# speedup vs baseline: 1.0947x; 1.0947x over previous
"""CEP loss kernel for Trainium2: loss = -sum(d1 * log(d2 + eps)).

Full inputs [4096, 4096] f32 are sharded row-wise across 8 NeuronCores
(512 rows each).  Per core the shard streams as 8 pieces of [128, 2048]
(1 MiB DMAs on the HWDGE queue):
  - ScalarE: t2 <- ln(d2 + eps) in place (+eps rides the activation bias)
  - VectorE: t2 <- d1 * t2 in place (fp32 tensor_tensor, 1x)
  - reduce along free dim to acc[:, piece], alternating engines per piece
    (even: ScalarE activation-Copy accum_out, odd: VectorE tensor_reduce)
    so neither engine becomes the critical path
Each core DMAs its [128, 8] partial-sum tile out; the host sums and
negates.  DMA (~16.8 MB/core at ~360-410 GB/s observed) is the
bottleneck: ACT ~23us and DVE ~26us busy both fit inside the ~41us DMA
window, and the post-DMA tail is only one piece of TT+reduce (~4us).
"""

import numpy as np

import concourse.bacc as bacc
import concourse.mybir as mybir
import concourse.tile as tile
from concourse.bass_utils import run_bass_kernel_spmd

N = 4096
N_CORES = 8
ROWS_PER_CORE = N // N_CORES  # 512
P = 128
N_TILES = ROWS_PER_CORE // P  # 4 row groups
PIECE_FD = 4096  # max piece width == SBUF tile width
MM_FD = 512  # one PSUM bank of fp32
EPS = 1e-5

# (row_tile, col_start, width) pieces: steady-state full-width 4096
# (2 MiB DMAs), tapered at the end so the post-DMA compute tail is short
_PIECES = []
for _i in range(N_TILES):
    widths = [4096]
    if _i == N_TILES - 1:
        widths = [2048, 1024, 512, 512]
    _c = 0
    for _w in widths:
        _PIECES.append((_i, _c, _w))
        _c += _w
    assert _c == N
N_PIECES = len(_PIECES)

_NC_CACHE = {}


def _build_nc():
    nc = bacc.Bacc(
        "TRN2", target_bir_lowering=False, debug=False, num_devices=N_CORES
    )
    d1 = nc.dram_tensor(
        "d1", [ROWS_PER_CORE, N], mybir.dt.float32, kind="ExternalInput"
    )
    d2 = nc.dram_tensor(
        "d2", [ROWS_PER_CORE, N], mybir.dt.float32, kind="ExternalInput"
    )
    out = nc.dram_tensor("partial", [1, 1], mybir.dt.float32, kind="ExternalOutput")
    d1t = d1.rearrange("(n p) m -> n p m", p=P)
    d2t = d2.rearrange("(n p) m -> n p m", p=P)

    with tile.TileContext(nc) as tc:
        with (
            tc.tile_pool(name="p1", bufs=4) as p1,
            tc.tile_pool(name="p2", bufs=4) as p2,
            tc.tile_pool(name="pprod", bufs=4) as pprod,
            tc.tile_pool(name="paux", bufs=1) as paux,
            tc.tile_pool(name="psum", bufs=1, space="PSUM") as psum_pool,
        ):
            bias = paux.tile([P, 1], mybir.dt.float32)
            nc.vector.memset(bias[:], EPS)
            ones = paux.tile([P, 1], mybir.dt.bfloat16)
            nc.vector.memset(ones[:], 1.0)
            colsum = psum_pool.tile([1, MM_FD], mybir.dt.float32)
            for k, (i, c0, w) in enumerate(_PIECES):
                fs = slice(c0, c0 + w)
                t1 = p1.tile([P, PIECE_FD], mybir.dt.float32, tag="t1")
                t2 = p2.tile([P, PIECE_FD], mybir.dt.float32, tag="t2")
                prod = pprod.tile([P, PIECE_FD], mybir.dt.bfloat16, tag="prod")
                nc.sync.dma_start(t2[:, :w], d2t[i][:, fs])
                nc.sync.dma_start(t1[:, :w], d1t[i][:, fs])
                # t2 <- ln(d2 + eps), in place on ScalarE
                nc.scalar.activation(
                    t2[:, :w],
                    t2[:, :w],
                    mybir.ActivationFunctionType.Ln,
                    bias=bias[:, :],
                )
                # prod <- d1 * t2 on VectorE, cast to bf16 on the write
                nc.vector.tensor_mul(prod[:, :w], t1[:, :w], t2[:, :w])
                # column sums on the otherwise-idle TensorE (native bf16
                # matmul), every chunk accumulating into one PSUM bank
                for j in range(w // MM_FD):
                    nc.tensor.matmul(
                        colsum[:, :],
                        ones[:, 0:1],
                        prod[:, j * MM_FD : (j + 1) * MM_FD],
                        start=(k == 0 and j == 0),
                        stop=(k == N_PIECES - 1 and j == w // MM_FD - 1),
                    )
            total = paux.tile([1, 1], mybir.dt.float32)
            nc.vector.tensor_reduce(
                total[:, 0:1],
                colsum[:],
                axis=mybir.AxisListType.X,
                op=mybir.AluOpType.add,
            )
            nc.sync.dma_start(out[:], total[:])
    nc.compile()
    return nc


def _get_nc():
    if "nc" not in _NC_CACHE:
        _NC_CACHE["nc"] = _build_nc()
    return _NC_CACHE["nc"]


def run_spmd(in_maps, **kwargs):
    """Run the SPMD kernel; returns BassKernelResults (test harness passes
    trace=True kwargs for profiling)."""
    return run_bass_kernel_spmd(
        _get_nc(), in_maps, core_ids=list(range(N_CORES)), **kwargs
    )


def make_in_maps(distribution1, distribution2):
    d1 = np.asarray(distribution1, dtype=np.float32)
    d2 = np.asarray(distribution2, dtype=np.float32)
    in_maps = []
    for c in range(N_CORES):
        sl = slice(c * ROWS_PER_CORE, (c + 1) * ROWS_PER_CORE)
        in_maps.append(
            {
                "d1": np.ascontiguousarray(d1[sl]),
                "d2": np.ascontiguousarray(d2[sl]),
            }
        )
    return in_maps


def reduce_outputs(results):
    total = np.float64(0.0)
    for r in results:
        total += np.float64(r["partial"].sum(dtype=np.float64))
    return np.asarray([-total], dtype=np.float32)


def kernel(distribution1, distribution2):
    in_maps = make_in_maps(distribution1, distribution2)
    res = run_spmd(in_maps)
    return reduce_outputs(res.results)
